# revision 23
# baseline (speedup 1.0000x reference)
"""Trainium2 Bass kernel for nn_DAN_46943992545473 (segment_reduce).

reference:
  x = concat(emb_table[seq], pos_table[pos], axis=2)          # [B, S, 100]
  pooled = (x * (s < seq_length)).sum(s) / seq_length         # [B, 100]
  out = MLP(pooled)  (relu x3, linear)                        # [B, 2]

Strategy (8 cores, data-parallel on batch: 256 rows/core):
  The masked-mean of gathered embedding rows is a sparse-matrix product:
     pooled_emb = C @ emb_table,   C[b, v] = #{s < L_b : seq[b,s] = v}
  computed per (core, batch-half of 128) over a host-compacted vocab.
  The kernel is HBM-bound, so C's footprint is cut ~2.3x by column
  bucketing: ~77% of compacted vocab rows are referenced by batches of
  only ONE 32-batch subgroup; those rows ship 32 count columns and run
  as col-tiled matmuls into the psum partition slice 32j:32j+32
  (tile_position=(0,32j)); the rest ship full 128 columns. A host-side
  batch permutation (snake-deal by seq_length over the 64 subgroups)
  equalizes bucket sizes across cores/halves so the compile-time
  padding is tight; the output is un-permuted on the host. C ships as
  fp8e4 raw counts (exact: counts <= 16; bf16 fallback otherwise), emb
  rows in bf16 (fp8 emb fails the 2e-2 gate - measured 2.8e-2).
  ct/emb chunks alternate between the two HWDGE rings (sync & scalar)
  so both rings carry ~equal bytes and the lockstep consumer is never
  gated by one lagging ring. The pos contribution rides separate
  matmuls from a tiny table (pos = arange). Batch-half 0's MLP is
  injected layer-by-layer between half 1's stream chunks; each half's
  [2,128] output slice is stored as soon as its final layer finishes.
  The 1/L scale rides the psum->SBUF ACT copy; PE transposes restore
  [dim, batch] for the MLP. The NEFF is compiled per (dtype mode,
  bucket block counts) and cached.
"""
import numpy as np
import ml_dtypes

import concourse.bacc as bacc
import concourse.bass as bass
import concourse.tile as tile
import concourse.mybir as mybir
from concourse.bass_utils import run_bass_kernel_spmd

# problem shapes (hardcoded per contract)
B, S = 2048, 512
VOCAB, MAXPOS = 50000, 512
DE = 50
DIN, H, OUT = 100, 512, 2
NCORES = 8
BL = B // NCORES            # 256 batches per core
NSUB = 4                    # 32-batch subgroups per half

NBS = MAXPOS // 128         # 4 pos blocks
NBH = BL // 128             # batch halves (2)

F32 = mybir.dt.float32
BF16 = mybir.dt.bfloat16
F8 = mybir.dt.float8e4
Act = mybir.ActivationFunctionType
Alu = mybir.AluOpType


def mix_chunks(nbvmix):
    """Small head chunk (quick PE start), then the rest."""
    if nbvmix <= 16:
        return (nbvmix,)
    return (12, nbvmix - 12)


def build_nc(mode, nbvmix, nbv32):
    fp8 = mode == "fp8"
    ctdt = F8 if fp8 else BF16
    ctsz = 1 if fp8 else 2
    nbv_e = nbvmix + NSUB * nbv32          # emb blocks per half
    fc = nbvmix * 128 + NSUB * nbv32 * 32  # ct cols per half (elements)
    mixch = mix_chunks(nbvmix)
    chmax = max(max(mixch), nbv32)

    nc = bacc.Bacc("TRN2", target_bir_lowering=False, debug=False)
    d_emb = nc.dram_tensor("embp", [128, NBH * nbv_e * DE], BF16,
                           kind="ExternalInput")
    d_ct = nc.dram_tensor("ctp", [128, NBH * fc], ctdt, kind="ExternalInput")
    # fused small constants (see _prep_shared/_run for the packing)
    d_pc = nc.dram_tensor("pc", [128, NBS * (DE + BL)], BF16,
                          kind="ExternalInput")
    d_w1f = nc.dram_tensor("w1f", [128, H + NBS * OUT], BF16,
                           kind="ExternalInput")
    d_w23 = nc.dram_tensor("w23", [128, NBS * 2 * H], BF16,
                           kind="ExternalInput")
    d_bias = nc.dram_tensor("biasf", [128, 15], F32, kind="ExternalInput")
    d_id = nc.dram_tensor("ident", [128, 128], F32, kind="ExternalInput")
    d_out = nc.dram_tensor("outT", [OUT, BL], F32, kind="ExternalOutput")

    emb_ap = d_emb.ap().rearrange("p (h k e) -> p h k e", h=NBH, e=DE)

    # per-half chunk plan: (kind, emb block offset, ct col offset, nblocks,
    # cols, psum col-slice j or None)
    plan = []
    eo, co = 0, 0
    for chb in mixch:
        plan.append(("mix", eo, co, chb, 128, None))
        eo += chb
        co += chb * 128
    for j in range(NSUB):
        plan.append(("b32", eo, co, nbv32, 32, j))
        eo += nbv32
        co += nbv32 * 32
    assert eo == nbv_e and co == fc

    with tile.TileContext(nc) as tc:
        with (
            tc.tile_pool(name="const", bufs=1) as cp,
            tc.tile_pool(name="strm", bufs=6) as sp,
            tc.tile_pool(name="mlp", bufs=1) as mp,
            tc.tile_pool(name="psum", bufs=1, space="PSUM") as qp,
        ):
            qs = [nc.sync, nc.scalar]   # ring alternation

            # ---- prefetch half-0 chunks 0+1 on both rings, then consts
            # (w23 split across rings; stream data beats the weights so
            # the first matmuls aren't queued behind 1MB of constants) ----
            pre01 = []
            for c in range(min(2, len(plan))):
                kind, eo, co, chb, cols, j = plan[c]
                et = sp.tile([128, chmax, DE], BF16, tag="et")
                qs[(c + 1) % 2].dma_start(et[:, 0:chb, :],
                                          emb_ap[:, 0, eo:eo + chb, :])
                tag = "ctm" if kind == "mix" else "ctb"
                shp = [128, chmax, 128] if kind == "mix" else [128, nbv32, 32]
                ct = sp.tile(shp, ctdt, tag=tag)
                qs[c % 2].dma_start(
                    ct[:, 0:chb, :],
                    d_ct.ap()[:, co:co + chb * cols]
                    .rearrange("p (k b) -> p k b", b=cols))
                pre01.append((et, ct))
            pct = cp.tile([128, NBS, DE + BL], BF16, tag="pct")
            nc.sync.dma_start(
                pct[:], d_pc.ap().rearrange("p (k f) -> p k f", f=DE + BL))
            biasf = cp.tile([128, 15], F32, tag="biasf")
            nc.scalar.dma_start(biasf[:], d_bias.ap())
            ident = cp.tile([128, 128], F32, tag="ident")
            nc.sync.dma_start(ident[:], d_id.ap())
            w1f = mp.tile([128, H + NBS * OUT], BF16, tag="w1f")
            nc.scalar.dma_start(w1f[:], d_w1f.ap())
            w23 = mp.tile([128, NBS, 2 * H], BF16, tag="w23")
            w23_ap = d_w23.ap().rearrange("p (k f) -> p k f", f=2 * H)
            nc.sync.dma_start(w23[:, 0:2, :], w23_ap[:, 0:2, :])
            nc.scalar.dma_start(w23[:, 2:NBS, :], w23_ap[:, 2:NBS, :])
            w1t = w1f[:, 0:H]
            wft = w1f[:, H:].rearrange("p (k o) -> p k o", o=OUT)
            w2t = w23[:, :, 0:H]
            w3t = w23[:, :, H:2 * H]
            bts = [biasf[:, 0:4], biasf[:, 4:8], biasf[:, 8:12]]
            rlt = biasf[:, 12:14]
            bft = biasf[0:OUT, 14:15]

            pooled = mp.tile([128, BL], BF16, tag="pooled")
            nc.vector.memset(pooled[:], 0.0)
            outT = mp.tile([OUT, BL], F32, tag="outT")
            pe = qp.tile([128, NBH * DE], F32, tag="pe")
            pes = [pe[:, 0:DE], pe[:, DE:2 * DE]]
            ppos = qp.tile([DE, BL], F32, tag="ppos")
            qctr = [0]   # ring alternation counter

            def emb_phase(h, pre=None, hooks=None):
                """One batch-half pass: mix chunks (full 128-col C blocks
                into pes[h]) then 4 col-bucket chunks (32-col C blocks
                into pes[h][32j:32j+32]). hooks[c] (e.g. an MLP layer of
                the other half) is emitted AFTER chunk c's matmuls, so a
                hook waiting on its inputs never blocks stream matmuls
                queued behind it in the PE FIFO."""
                last = len(plan) - 1
                for c, (kind, eo, co, chb, cols, j) in enumerate(plan):
                    qa = qs[(qctr[0] + h) % 2]
                    qb = qs[(qctr[0] + h + 1) % 2]
                    qctr[0] += 1
                    if pre is not None and c < len(pre):
                        et, ct = pre[c]
                    else:
                        et = sp.tile([128, chmax, DE], BF16, tag="et")
                        qa.dma_start(et[:, 0:chb, :],
                                     emb_ap[:, h, eo:eo + chb, :])
                        tag = "ctm" if kind == "mix" else "ctb"
                        shp = [128, chmax, 128] if kind == "mix" else \
                            [128, nbv32, 32]
                        ct = sp.tile(shp, ctdt, tag=tag)
                        qb.dma_start(
                            ct[:, 0:chb, :],
                            d_ct.ap()[:, h * fc + co:h * fc + co + chb * cols]
                            .rearrange("p (k b) -> p k b", b=cols))
                    for k in range(chb):
                        if kind == "mix":
                            nc.tensor.matmul(
                                pes[h][:], ct[:, k, :], et[:, k, :],
                                start=(eo + k == 0), stop=False,
                                skip_group_check=True)
                        else:
                            nc.tensor.matmul(
                                pes[h][32 * j:32 * j + 32, :],
                                ct[:, k, :], et[:, k, :],
                                start=False, stop=(c == last and k == chb - 1),
                                tile_position=(0, 32 * j),
                                skip_group_check=True)
                    if hooks and c in hooks:
                        hooks[c]()
                    yield c

            def junk_mms(n):
                """Dependency-free matmuls that keep the PE HAM activity
                window busy through otherwise-idle semaphore waits, so the
                tail MLP doesn't run at the 1.2 GHz throttled clock. They
                scribble over the ppos psum tile, whose last reader (the
                mid-stream pos copy) is long done."""
                junkps = qp.tile([DE, BL], F32, tag="ppos")
                for _ in range(n):
                    nc.tensor.matmul(junkps[:, 0:64], w23[0:128, 0, 0:DE],
                                     w23[0:128, 0, 64:128],
                                     start=True, stop=True,
                                     skip_group_check=True)

            hes = [mp.tile([128, DE], F32, tag=f"he{h}", name=f"he{h}")
                   for h in range(NBH)]
            tr2 = qp.tile([DE, NBH * 128], F32, tag="tr")
            trs = [tr2[:, 0:128], tr2[:, 128:256]]

            def head_scale(h, j, tail=False):
                """Scale one 32-batch slice of pes[h] into hes[h] (on DVE,
                or ACT for the tail slice, when scalar has no DMA issues
                left to block). Emitted right after bucket j's chain."""
                s = 32 * j
                if tail:
                    nc.scalar.activation(hes[h][s:s + 32, :],
                                         pes[h][s:s + 32, :], Act.Identity,
                                         bias=0.0,
                                         scale=rlt[s:s + 32, h:h + 1])
                else:
                    nc.vector.tensor_scalar(hes[h][s:s + 32, :],
                                            pes[h][s:s + 32, :],
                                            rlt[s:s + 32, h:h + 1], 0.0,
                                            op0=Alu.mult, op1=Alu.add)

            def head_fin(h, j, tail=False):
                """Transpose+copy for a slice whose scale ran a chunk ago,
                so the PE transpose never stalls the stream matmuls queued
                behind it (its input is long ready)."""
                o, s = h * 128, 32 * j
                nc.tensor.matmul(trs[h][:, s:s + 32], hes[h][s:s + 32, :],
                                 ident[s:s + 32, s:s + 32],
                                 is_transpose=True,
                                 tile_position=(s, 0),
                                 skip_group_check=True)
                if tail:
                    nc.scalar.copy(pooled[0:DE, o + s:o + s + 32],
                                   trs[h][:, s:s + 32])
                else:
                    nc.vector.tensor_copy(pooled[0:DE, o + s:o + s + 32],
                                          trs[h][:, s:s + 32])

            def mlp_pieces(h, warm=False):
                """Emitters for one batch-half MLP, one per layer, so the
                pieces can interleave with the other half's C stream.
                The interleaved half (warm=False) keeps every dependent
                op off the DMA-issuing scalar/sync queues (relus on DVE,
                final bias+store on GpSimd); the tail half (warm=True)
                splits relus ACT/DVE for latency and adds PE keep-warm."""
                o = h * 128
                state = {"hcur": pooled[:, o:o + 128]}

                def layer(li, wt, bt):
                    def emit():
                        if warm:
                            junk_mms(8)
                        hcur = state["hcur"]
                        houts = []
                        for m in range(H // 128):
                            ps = qp.tile([128, 128], F32, tag=f"h{m}")
                            if li == 0:
                                nc.tensor.matmul(
                                    ps[:], wt[:, m * 128:(m + 1) * 128],
                                    hcur, start=True, stop=True)
                            else:
                                for cc in range(H // 128):
                                    nc.tensor.matmul(
                                        ps[:],
                                        wt[:, cc, m * 128:(m + 1) * 128],
                                        hcur[cc][:], start=(cc == 0),
                                        stop=(cc == H // 128 - 1))
                            ht = mp.tile([128, 128], BF16,
                                         tag=f"a{li}m{m}h{h}")
                            if warm and m < 2:
                                nc.scalar.activation(ht[:], ps[:], Act.Relu,
                                                     bias=bt[:, m:m + 1])
                            else:
                                nc.vector.tensor_scalar(
                                    ht[:], ps[:], bt[:, m:m + 1], 0.0,
                                    op0=Alu.add, op1=Alu.max)
                            houts.append(ht)
                        state["hcur"] = houts
                    return emit

                def final():
                    hcur = state["hcur"]
                    pso = qp.tile([OUT, 128], F32, tag="out")
                    for cc in range(H // 128):
                        nc.tensor.matmul(pso[:], wft[:, cc, :], hcur[cc][:],
                                         start=(cc == 0),
                                         stop=(cc == H // 128 - 1))
                    if warm:
                        nc.scalar.activation(outT[0:OUT, o:o + 128], pso[:],
                                             Act.Identity, bias=bft[:, :1])
                        nc.sync.dma_start(d_out.ap()[:, o:o + 128],
                                          outT[0:OUT, o:o + 128])
                    else:
                        nc.vector.tensor_scalar(
                            outT[0:OUT, o:o + 128], pso[:], bft[:, 0:1],
                            0.0, op0=Alu.add, op1=Alu.add)
                        nc.gpsimd.dma_start(d_out.ap()[:, o:o + 128],
                                            outT[0:OUT, o:o + 128])
                return [layer(0, w1t, bts[0]), layer(1, w2t, bts[1]),
                        layer(2, w3t, bts[2]), final]

            # phase 0: half-0 C stream; pos chain rides along and its
            # pooled contribution is copied in mid-stream (both halves)
            def pos_chain():
                for k in range(NBS):
                    nc.tensor.matmul(ppos[:], pct[:, k, 0:DE],
                                     pct[:, k, DE:], start=(k == 0),
                                     stop=(k == NBS - 1))

            def pos_copies():
                nc.vector.tensor_copy(pooled[64:64 + DE, 0:BL], ppos[:])

            def seq(*fns):
                def run():
                    for f in fns:
                        f()
                return run
            nmix = len(mixch)

            def head_hooks(base, h, tail):
                """scale(j) fires at bucket j's chunk; fin(j) one chunk
                later; fin of the last bucket is left for the caller."""
                hk = {}
                for j in range(NSUB):
                    fns = [(lambda jj: lambda: head_scale(
                        h, jj, tail=(tail and jj == NSUB - 1)))(j)]
                    if j >= 1:
                        fns.append((lambda jj: lambda: head_fin(h, jj))(j - 1))
                    hk[base + j] = seq(*fns)
                return hk

            hooks0 = {1: pos_chain, 3: pos_copies}
            for c, fn in head_hooks(nmix, 0, tail=False).items():
                hooks0[c] = seq(hooks0.get(c, lambda: None), fn)
            for c in emb_phase(0, pre=pre01, hooks=hooks0):
                pass
            head_fin(0, NSUB - 1)
            # phase 1: half-1 C stream with half-0 MLP injected layer by
            # layer after the first chunks' matmuls; half-1 head slices
            # emitted per bucket as they complete
            pieces0 = mlp_pieces(0)
            hooks1 = dict(enumerate(pieces0))
            for c, fn in head_hooks(nmix, 1, tail=True).items():
                hooks1[c] = seq(hooks1.get(c, lambda: None), fn)
            for c in emb_phase(1, hooks=hooks1):
                pass
            junk_mms(20)
            head_fin(1, NSUB - 1, tail=True)
            for piece in mlp_pieces(1, warm=True):
                piece()

    nc.compile()
    return nc


_NC_CACHE = {}


def _pad_w1(w1):
    wp = np.zeros((128, H), np.float32)
    wp[0:DE] = w1[0:DE]
    wp[64:64 + DE] = w1[DE:DIN]
    return wp


def _blockify(a, nblk, dtype):
    """[nblk*128, F] row-major -> [128, nblk, F] SBUF-partition-major."""
    f = a.shape[1]
    return np.ascontiguousarray(
        a.reshape(nblk, 128, f).transpose(1, 0, 2)).astype(dtype)


def _prep_shared(pos_table, W1, b1, W2, b2, W3, b3, Wf, bf):
    bf16 = ml_dtypes.bfloat16
    w1f = np.concatenate(
        [_pad_w1(np.asarray(W1, np.float32)),
         np.asarray(Wf, np.float32).reshape(NBS, 128, OUT)
         .transpose(1, 0, 2).reshape(128, NBS * OUT)], axis=1)
    w23 = np.concatenate(
        [_blockify(np.asarray(W2, np.float32), NBS, np.float32),
         _blockify(np.asarray(W3, np.float32), NBS, np.float32)],
        axis=2).reshape(128, NBS * 2 * H)
    return {
        "w1f": w1f.astype(bf16),
        "w23": w23.astype(bf16),
        "ident": np.eye(128, dtype=np.float32),
        "_posp": _blockify(np.asarray(pos_table, np.float32), NBS, np.float32),
        "_b123": np.stack([np.asarray(x, np.float32).reshape(NBS, 128).T
                           for x in (b1, b2, b3)], axis=1).reshape(128, 12),
        "_bf": np.asarray(bf, np.float32).reshape(OUT),
    }


def _count_matrix(idx, mask, width, nb):
    """C.T: [width, nb] f32 with C[b, v] = #{s: mask[b,s] and idx[b,s]==v}."""
    b_of = np.broadcast_to(np.arange(nb)[:, None], idx.shape)
    flat = idx[mask].astype(np.int64) * nb + b_of[mask]
    cnt = np.bincount(flat, minlength=width * nb).astype(np.float32)
    return cnt.reshape(width, nb)


def _balance_perm(slen):
    """Snake-deal batches by length across the 64 (core, half, subgroup)
    slots so per-bucket unique-vocab counts are nearly equal."""
    order = np.argsort(slen, kind="stable")
    nsg = NCORES * NBH * NSUB
    members = [[] for _ in range(nsg)]
    for r, b in enumerate(order):
        rnd, pos = divmod(r, nsg)
        g = pos if rnd % 2 == 0 else nsg - 1 - pos
        members[g].append(b)
    return np.concatenate([np.asarray(m, np.int64) for m in members])


def _classify(seqs, mask):
    """For one 128-batch half: unique vocab rows split into 4 exclusive
    32-batch-subgroup buckets + mix; returns (buckets list, mix array)."""
    b_of = np.broadcast_to(np.arange(128)[:, None], seqs.shape)
    tv = seqs[mask]
    tg = (b_of[mask] // 32).astype(np.uint8)
    gmask = np.zeros(VOCAB, np.uint8)
    np.bitwise_or.at(gmask, tv, np.uint8(1) << tg)
    uniq = np.unique(tv)
    gm = gmask[uniq]
    buckets = [uniq[gm == (1 << j)] for j in range(NSUB)]
    is_mix = (gm & (gm - 1)) != 0
    return buckets, uniq[is_mix]


def _run(inputs, trace=False):
    seq_full = np.asarray(inputs["seq"], np.int64)
    pos_full = np.asarray(inputs["pos"], np.int64)
    slen_full = np.asarray(inputs["seq_length"], np.int64)
    emb_f32 = np.asarray(inputs["emb_table"], np.float32)
    bf16 = ml_dtypes.bfloat16

    perm = _balance_perm(slen_full)
    seq = seq_full[perm]
    pos_i = pos_full[perm]
    slen = slen_full[perm]

    shared = _prep_shared(
        inputs["pos_table"], inputs["W1"], inputs["b1"],
        inputs["W2"], inputs["b2"], inputs["W3"], inputs["b3"],
        inputs["Wf"], inputs["bf"])
    hidden = {k: shared.pop(k) for k in list(shared) if k.startswith("_")}

    smask = np.arange(S)[None, :] < slen[:, None]       # [B, S]
    rl_all = (1.0 / slen).astype(np.float32)

    # classify every (core, half): 4 exclusive 32-subgroup buckets + mix
    cls = [[None] * NBH for _ in range(NCORES)]
    cposs = []
    for i in range(NCORES):
        sl = slice(i * BL, (i + 1) * BL)
        cposs.append(_count_matrix(pos_i[sl], smask[sl], MAXPOS, BL))
        for h in range(NBH):
            slb = slice(i * BL + h * 128, i * BL + (h + 1) * 128)
            cls[i][h] = _classify(seq[slb], smask[slb])
    nbv32 = max(-(-len(bk) // 128)
                for row in cls for bks, _ in row for bk in bks)
    nbvmix = max(-(-len(mx) // 128) for row in cls for _, mx in row)
    nbv_e = nbvmix + NSUB * nbv32
    fc = nbvmix * 128 + NSUB * nbv32 * 32

    # per (core, half): section-ordered row list, counts, compacted emb
    maxcnt = 0.0
    ctps, embps = [], []
    for i in range(NCORES):
        ct_halves, emb_halves = [], []
        for h in range(NBH):
            slb = slice(i * BL + h * 128, i * BL + (h + 1) * 128)
            buckets, mix = cls[i][h]
            sq, mk = seq[slb], smask[slb]
            # mix section: full 128 count columns
            remap = np.full(VOCAB, -1, np.int64)
            remap[mix] = np.arange(len(mix))
            mused = remap[sq] >= 0
            cmix = _count_matrix(remap[sq], mk & mused, nbvmix * 128, 128)
            parts = [cmix.reshape(nbvmix, 128, 128)]
            rows = np.zeros((nbv_e * 128, DE), np.float32)
            rows[0:len(mix)] = emb_f32[mix]
            ro = nbvmix * 128
            for j in range(NSUB):
                bk = buckets[j]
                sqj = sq[32 * j:32 * j + 32]
                mkj = mk[32 * j:32 * j + 32]
                remap = np.full(VOCAB, -1, np.int64)
                remap[bk] = np.arange(len(bk))
                used = remap[sqj] >= 0
                cb = _count_matrix(remap[sqj], mkj & used, nbv32 * 128, 32)
                parts.append(cb.reshape(nbv32, 128, 32))
                rows[ro:ro + len(bk)] = emb_f32[bk]
                ro += nbv32 * 128
            maxcnt = max(maxcnt, max(p.max() for p in parts))
            # ct layout per half: [128p, mix nbvmix*128 ++ 4x nbv32*32]
            ct_halves.append(np.concatenate(
                [p.transpose(1, 0, 2).reshape(128, -1) for p in parts],
                axis=1))
            emb_halves.append(
                rows.reshape(nbv_e, 128, DE).transpose(1, 0, 2)
                .reshape(128, nbv_e * DE))
        ctps.append(np.concatenate(ct_halves, axis=1))
        embps.append(np.concatenate(emb_halves, axis=1).astype(bf16))

    # counts are fp8e4-exact up to 16; fall back to bf16 otherwise
    mode = "fp8" if maxcnt <= 16 else "bf16"
    ctdt = ml_dtypes.float8_e4m3 if mode == "fp8" else bf16
    key = (mode, nbvmix, nbv32)
    if key not in _NC_CACHE:
        _NC_CACHE[key] = build_nc(mode, nbvmix, nbv32)
    nc = _NC_CACHE[key]

    in_maps = []
    for i in range(NCORES):
        sl = slice(i * BL, (i + 1) * BL)
        rl = rl_all[sl]
        m = dict(shared)
        m["embp"] = embps[i]
        m["ctp"] = np.ascontiguousarray(ctps[i]).astype(ctdt)
        cpos = cposs[i] * rl[None, :]
        m["pc"] = np.concatenate(
            [hidden["_posp"], _blockify(cpos, NBS, np.float32)],
            axis=2).reshape(128, NBS * (DE + BL)).astype(bf16)
        biasf = np.zeros((128, 15), np.float32)
        biasf[:, 0:12] = hidden["_b123"]
        biasf[:, 12:14] = rl.reshape(NBH, 128).T
        biasf[0:OUT, 14] = hidden["_bf"]
        m["biasf"] = biasf
        in_maps.append(m)

    res = run_bass_kernel_spmd(nc, in_maps, core_ids=list(range(NCORES)),
                               trace=trace)
    outp = np.concatenate([res.results[i]["outT"].T for i in range(NCORES)],
                          axis=0)
    out = np.empty_like(outp)
    out[perm] = outp
    return np.ascontiguousarray(out, dtype=np.float32), res


def kernel(emb_table, pos_table, W1, b1, W2, b2, W3, b3, Wf, bf,
           seq, seq_length, pos):
    out, _ = _run(dict(emb_table=emb_table, pos_table=pos_table, W1=W1, b1=b1,
                       W2=W2, b2=b2, W3=W3, b3=b3, Wf=Wf, bf=bf, seq=seq,
                       seq_length=seq_length, pos=pos))
    return out
